# Initial kernel scaffold
#
"""Trainium2 Bass kernel for nn_Attention_42125039239602.

8-head attention with additive bias, sigmoid gating, and output projection.
Sharding: one head per NeuronCore (tensor parallel). Each core computes its
head's attention plus its slice of the gated output projection; the host sums
the 8 row-parallel partial outputs and adds bo.

Math per core (head h):
    qT = (Wq_h^T x^T) * scale          [64, seq]   (scale folded into Wq host-side)
    kT = Wk_h^T x^T                    [64, seq]
    gT = sigmoid(Wg_h^T x^T + bg_h)    [64, seq]
    v  = (Wv_h^T x^T)^T (PE transpose) [seq, 65]   col 64 = 1.0 (colsum trick)
    S^T tile = kT_chunk^T qT_chunk     [128k, 512q]
    P^T = exp(S^T) * expB^T            (expB = exp(bias) precomputed on host)
    otT[65, q] = sum_k v_aug_chunk^T P^T_chunk     row 64 = softmax denominator
    ogT = otT[0:64] * gT               (unnormalized)
    out_tile[128q, 512] = (ogT_chunk^T wo) * (1/denom)[q]   per-partition scale
"""

import os
import numpy as np

HEADS = 8
DH = 64
B = 2
N = 2048
D = 512
SEQ = B * N  # 4096
SCALE = DH ** -0.5

_CACHE = {}


def build_nc(reps: int = 1, dtype_mm: str = "float32r"):
    """Build the single-core Bass program (SPMD across 8 cores)."""
    import concourse.bass as bass  # noqa: F401
    import concourse.mybir as mybir
    from concourse import bacc
    from concourse.tile import TileContext
    from concourse.masks import make_identity

    f32 = mybir.dt.float32
    fmm = getattr(mybir.dt, dtype_mm)
    AF = mybir.ActivationFunctionType

    nc = bacc.Bacc("TRN2", target_bir_lowering=False, debug=False)

    xT_d = nc.dram_tensor("xT", [D, SEQ], fmm, kind="ExternalInput")
    expBT_d = nc.dram_tensor("expBT", [N, N], f32, kind="ExternalInput")
    wqk_d = nc.dram_tensor("wqk", [D, 128], fmm, kind="ExternalInput")
    wgv_d = nc.dram_tensor("wgv", [D, 128], fmm, kind="ExternalInput")
    bg_d = nc.dram_tensor("bg", [DH, 1], f32, kind="ExternalInput")
    wo_d = nc.dram_tensor("wo", [DH, D], fmm, kind="ExternalInput")
    out_d = nc.dram_tensor("out", [SEQ, D], f32, kind="ExternalOutput")

    with TileContext(nc) as tc:
        with (
            tc.tile_pool(name="persist", bufs=1) as persist,
            tc.tile_pool(name="work", bufs=1) as work,
        ):
            # ---- weights / constants (loaded once) ----
            wqk_s = persist.tile([128, 4, 128], fmm)
            nc.sync.dma_start(out=wqk_s, in_=wqk_d.ap().rearrange("(c p) m -> p c m", p=128))
            wgv_s = persist.tile([128, 4, 128], fmm)
            nc.sync.dma_start(out=wgv_s, in_=wgv_d.ap().rearrange("(c p) m -> p c m", p=128))
            wo_s = persist.tile([DH, D], fmm)
            nc.sync.dma_start(out=wo_s, in_=wo_d.ap())
            bg_s = persist.tile([DH, 1], f32)
            nc.sync.dma_start(out=bg_s, in_=bg_d.ap())
            ident = persist.tile([128, 128], f32)
            make_identity(nc, ident)

            xT_s = persist.tile([128, 4, SEQ], fmm)
            nc.sync.dma_start(out=xT_s, in_=xT_d.ap().rearrange("(c p) m -> p c m", p=128))

            for rep in range(reps):
                # ---- per-iteration activation tensors ----
                qT = work.tile([DH, SEQ], fmm, tag=f"qT{rep}")
                kT = work.tile([DH, SEQ], fmm, tag=f"kT{rep}")
                gT = work.tile([DH, SEQ], f32, tag=f"gT{rep}")
                vN = work.tile([128, 32, 65], fmm, tag=f"vN{rep}")
                otT = work.tile([65, SEQ], f32, tag=f"otT{rep}")
                ogT = work.tile([DH, SEQ], fmm, tag=f"ogT{rep}")
                recip = work.tile([128, 32], f32, tag=f"recip{rep}")

                nc.vector.memset(vN[:, :, 64:65], 1.0)

                # ---- projections ----
                with (
                    tc.tile_pool(name="pp", bufs=4, space="PSUM") as pp,
                    tc.tile_pool(name="vsp", bufs=3) as vsp,
                    tc.tile_pool(name="vtp", bufs=3, space="PSUM") as vtp,
                ):
                    for sc in range(SEQ // 512):
                        s0 = sc * 512
                        ps = pp.tile([128, 512], f32, tag="ps_qk")
                        for dc in range(4):
                            nc.tensor.matmul(
                                ps, wqk_s[:, dc, :], xT_s[:, dc, s0:s0 + 512],
                                start=(dc == 0), stop=(dc == 3),
                            )
                        nc.vector.tensor_copy(qT[:, s0:s0 + 512], ps[0:DH, :])
                        nc.vector.tensor_copy(kT[:, s0:s0 + 512], ps[DH:128, :])

                        ps2 = pp.tile([128, 512], f32, tag="ps_gv")
                        for dc in range(4):
                            nc.tensor.matmul(
                                ps2, wgv_s[:, dc, :], xT_s[:, dc, s0:s0 + 512],
                                start=(dc == 0), stop=(dc == 3),
                            )
                        nc.scalar.activation(
                            gT[:, s0:s0 + 512], ps2[0:DH, :], AF.Sigmoid,
                            bias=bg_s[:, 0:1],
                        )
                        vst = vsp.tile([DH, 512], f32, tag="vst")
                        nc.vector.tensor_copy(vst, ps2[DH:128, :])
                        for j in range(4):
                            t = sc * 4 + j
                            tp = vtp.tile([128, DH], f32, tag="vtp")
                            nc.tensor.transpose(tp, vst[:, j * 128:(j + 1) * 128], ident[0:DH, 0:DH])
                            nc.vector.tensor_copy(vN[:, t, 0:DH], tp)

                # ---- attention (2 query-halves to fit PSUM) ----
                for qg in range(2):
                    with (
                        tc.tile_pool(name="otp", bufs=4, space="PSUM") as otp,
                        tc.tile_pool(name="ssp", bufs=3, space="PSUM") as ssp,
                        tc.tile_pool(name="ebp", bufs=3) as ebp,
                        tc.tile_pool(name="esp", bufs=3) as esp,
                        tc.tile_pool(name="ptp", bufs=3) as ptp,
                    ):
                        ots = {}
                        for qc2 in range(2):
                            for b in range(2):
                                ots[(qc2, b)] = otp.tile([65, 512], f32, tag=f"ot{qc2}{b}")
                        for kc in range(16):
                            for qc2 in range(2):
                                qc = qg * 2 + qc2
                                bt = ebp.tile([128, 512], f32, tag="bt")
                                nc.sync.dma_start(
                                    out=bt,
                                    in_=expBT_d.ap()[kc * 128:(kc + 1) * 128, qc * 512:(qc + 1) * 512],
                                )
                                for b in range(2):
                                    q0 = b * N + qc * 512
                                    k0 = b * N + kc * 128
                                    sp = ssp.tile([128, 512], f32, tag="sp")
                                    nc.tensor.matmul(
                                        sp, kT[:, k0:k0 + 128], qT[:, q0:q0 + 512],
                                        start=True, stop=True,
                                    )
                                    es = esp.tile([128, 512], fmm, tag="es")
                                    nc.scalar.activation(es, sp, AF.Exp)
                                    pt = ptp.tile([128, 512], fmm, tag="pt")
                                    nc.vector.tensor_mul(pt, es, bt)
                                    nc.tensor.matmul(
                                        ots[(qc2, b)], vN[:, b * 16 + kc, :], pt,
                                        start=(kc == 0), stop=(kc == 15),
                                    )
                        # drain ot psum -> otT / ogT
                        for qc2 in range(2):
                            for b in range(2):
                                qc = qg * 2 + qc2
                                q0 = b * N + qc * 512
                                nc.vector.tensor_copy(otT[:, q0:q0 + 512], ots[(qc2, b)])
                                nc.vector.tensor_mul(
                                    ogT[:, q0:q0 + 512], otT[0:DH, q0:q0 + 512], gT[:, q0:q0 + 512]
                                )

                # ---- final projection + normalization ----
                with (
                    tc.tile_pool(name="fpp", bufs=4, space="PSUM") as fpp,
                    tc.tile_pool(name="ctp", bufs=2, space="PSUM") as ctp,
                    tc.tile_pool(name="osb", bufs=3) as osb,
                ):
                    for t in range(32):
                        q0 = t * 128
                        tp = ctp.tile([128, 65], f32, tag="ctp")
                        nc.tensor.transpose(tp, otT[:, q0:q0 + 128], ident[0:65, 0:65])
                        nc.vector.reciprocal(recip[:, t:t + 1], tp[:, 64:65])
                        fp = fpp.tile([128, 512], f32, tag="fp")
                        nc.tensor.matmul(fp, ogT[:, q0:q0 + 128], wo_s, start=True, stop=True)
                        ob = osb.tile([128, 512], f32, tag="ob")
                        if t % 2 == 0:
                            nc.scalar.activation(
                                ob, fp, AF.Copy, scale=recip[:, t:t + 1]
                            )
                        else:
                            nc.vector.tensor_scalar_mul(ob, fp, recip[:, t:t + 1])
                        nc.sync.dma_start(out=out_d.ap()[q0:q0 + 128, :], in_=ob)

    nc.compile()
    return nc


def make_in_maps(x, attn_bias, Wq, Wkv, Wo, bo, Wg, bg):
    x = np.asarray(x, dtype=np.float32)
    attn_bias = np.asarray(attn_bias, dtype=np.float32)
    Wq = np.asarray(Wq, dtype=np.float32)
    Wkv = np.asarray(Wkv, dtype=np.float32)
    Wo = np.asarray(Wo, dtype=np.float32)
    Wg = np.asarray(Wg, dtype=np.float32)
    bg = np.asarray(bg, dtype=np.float32)

    xT = np.ascontiguousarray(x.reshape(SEQ, D).T)
    Wk = Wkv[:, :HEADS * DH]
    Wv = Wkv[:, HEADS * DH:]
    in_maps = []
    for h in range(HEADS):
        sl = slice(h * DH, (h + 1) * DH)
        wqk = np.ascontiguousarray(
            np.concatenate([Wq[:, sl] * SCALE, Wk[:, sl]], axis=1))
        wgv = np.ascontiguousarray(
            np.concatenate([Wg[:, sl], Wv[:, sl]], axis=1))
        expBT = np.exp(np.ascontiguousarray(attn_bias[0, h].T))
        in_maps.append({
            "xT": xT,
            "expBT": expBT,
            "wqk": wqk,
            "wgv": wgv,
            "bg": np.ascontiguousarray(bg[sl].reshape(DH, 1)),
            "wo": np.ascontiguousarray(Wo[sl, :]),
        })
    return in_maps


def kernel(x, attn_bias, Wq, Wkv, Wo, bo, Wg, bg):
    from concourse.bass_utils import run_bass_kernel_spmd

    if "nc" not in _CACHE:
        _CACHE["nc"] = build_nc(reps=int(os.environ.get("KERNEL_REPS", "1")))
    nc = _CACHE["nc"]

    in_maps = make_in_maps(x, attn_bias, Wq, Wkv, Wo, bo, Wg, bg)
    res = run_bass_kernel_spmd(nc, in_maps, core_ids=list(range(HEADS)))
    out = np.zeros((SEQ, D), dtype=np.float64)
    for h in range(HEADS):
        out += res.results[h]["out"]
    out += np.asarray(bo, dtype=np.float64)
    return out.astype(np.float32).reshape(B, N, D)


# revision 5
# speedup vs baseline: 6.1454x; 6.1454x over previous
"""Trainium2 Bass kernel for nn_Attention_42125039239602.

8-head attention with additive bias, sigmoid gating, and output projection.
Sharding: one head per NeuronCore (tensor parallel). Each core computes its
head's attention plus its slice of the gated output projection; the host sums
the 8 row-parallel partial outputs and adds bo.

Math per core (head h):
    qT = (Wq_h^T x^T) * scale          [64, seq]   (scale folded into Wq host-side)
    kT = Wk_h^T x^T                    [64, seq]
    gT = sigmoid(Wg_h^T x^T + bg_h)    [64, seq]
    v  = (Wv_h^T x^T)^T (PE transpose) [seq, 65]   col 64 = 1.0 (colsum trick)
    S^T tile = kT_chunk^T qT_chunk     [128k, 512q]
    P^T = exp(S^T) * expB^T            (expB = exp(bias) precomputed on host)
    otT[65, q] = sum_k v_aug_chunk^T P^T_chunk     row 64 = softmax denominator
    ogT = otT[0:64] * gT               (unnormalized)
    out_tile[128q, 512] = (ogT_chunk^T wo) * (1/denom)[q]   per-partition scale
"""

import os
import numpy as np

HEADS = 8
DH = 64
B = 2
N = 2048
D = 512
SEQ = B * N  # 4096
SCALE = DH ** -0.5

_CACHE = {}


def build_nc(reps: int = 1, dtype_mm: str = "float32r"):
    """Build the single-core Bass program (SPMD across 8 cores)."""
    import concourse.bass as bass  # noqa: F401
    import concourse.mybir as mybir
    from concourse import bacc
    from concourse.tile import TileContext
    from concourse.masks import make_identity

    f32 = mybir.dt.float32
    fmm = getattr(mybir.dt, dtype_mm)
    AF = mybir.ActivationFunctionType

    nc = bacc.Bacc("TRN2", target_bir_lowering=False, debug=False)

    xT_d = nc.dram_tensor("xT", [D, SEQ], fmm, kind="ExternalInput")
    expBT_d = nc.dram_tensor("expBT", [N, N], f32, kind="ExternalInput")
    wqk_d = nc.dram_tensor("wqk", [D, 128], fmm, kind="ExternalInput")
    wgv_d = nc.dram_tensor("wgv", [D, 128], fmm, kind="ExternalInput")
    bg_d = nc.dram_tensor("bg", [DH, 1], f32, kind="ExternalInput")
    wo_d = nc.dram_tensor("wo", [DH, D], fmm, kind="ExternalInput")
    out_d = nc.dram_tensor("out", [SEQ, D], f32, kind="ExternalOutput")

    with TileContext(nc) as tc:
        with (
            tc.tile_pool(name="persist", bufs=1) as persist,
            tc.tile_pool(name="work", bufs=1) as work,
        ):
            # ---- weights / constants (loaded once) ----
            wqk_s = persist.tile([128, 4, 128], fmm)
            nc.sync.dma_start(out=wqk_s, in_=wqk_d.ap().rearrange("(c p) m -> p c m", p=128))
            wgv_s = persist.tile([128, 4, 128], fmm)
            nc.sync.dma_start(out=wgv_s, in_=wgv_d.ap().rearrange("(c p) m -> p c m", p=128))
            wo_s = persist.tile([DH, D], fmm)
            nc.sync.dma_start(out=wo_s, in_=wo_d.ap())
            bg_s = persist.tile([DH, 1], f32)
            nc.sync.dma_start(out=bg_s, in_=bg_d.ap())
            ident = persist.tile([128, 128], f32)
            make_identity(nc, ident)

            xT_s = persist.tile([128, 4, SEQ], fmm)
            nc.sync.dma_start(out=xT_s, in_=xT_d.ap().rearrange("(c p) m -> p c m", p=128))

            for rep in range(reps):
                # ---- per-iteration activation tensors ----
                qT = work.tile([DH, SEQ], fmm, tag="qT")
                kT = work.tile([DH, SEQ], fmm, tag="kT")
                gT = work.tile([DH, SEQ], f32, tag="gT")
                vN = work.tile([128, 32, 65], fmm, tag="vN")
                otT = work.tile([65, SEQ], f32, tag="otT")
                ogT = work.tile([DH, SEQ], fmm, tag="ogT")
                recip = work.tile([128, 32], f32, tag="recip")

                ones_t = work.tile([128, 32], f32, tag="ones", name="ones_t")
                nc.vector.memset(ones_t, 1.0)
                nc.vector.tensor_copy(vN[:, :, 64:65].rearrange("p a b -> p (a b)"), ones_t)

                # ---- projections ----
                with (
                    tc.tile_pool(name="pp", bufs=2, space="PSUM") as pp,
                    tc.tile_pool(name="vsp", bufs=3) as vsp,
                    tc.tile_pool(name="vtp", bufs=3, space="PSUM") as vtp,
                ):
                    for sc in range(SEQ // 512):
                        s0 = sc * 512
                        ps = pp.tile([128, 512], f32, tag="ps_qk")
                        for dc in range(4):
                            nc.tensor.matmul(
                                ps, wqk_s[:, dc, :], xT_s[:, dc, s0:s0 + 512],
                                start=(dc == 0), stop=(dc == 3),
                            )
                        nc.vector.tensor_copy(qT[:, s0:s0 + 512], ps[0:DH, :])
                        nc.vector.tensor_copy(kT[:, s0:s0 + 512], ps[DH:128, :])

                        ps2 = pp.tile([128, 512], f32, tag="ps_gv")
                        for dc in range(4):
                            nc.tensor.matmul(
                                ps2, wgv_s[:, dc, :], xT_s[:, dc, s0:s0 + 512],
                                start=(dc == 0), stop=(dc == 3),
                            )
                        nc.scalar.activation(
                            gT[:, s0:s0 + 512], ps2[0:DH, :], AF.Sigmoid,
                            bias=bg_s[:, 0:1],
                        )
                        vst = vsp.tile([DH, 512], f32, tag="vst")
                        nc.vector.tensor_copy(vst, ps2[DH:128, :])
                        for j in range(4):
                            t = sc * 4 + j
                            tp = vtp.tile([128, DH], f32, tag="vtp")
                            nc.tensor.transpose(tp, vst[:, j * 128:(j + 1) * 128], ident[0:DH, 0:DH])
                            nc.vector.tensor_copy(vN[:, t, 0:DH], tp)

                # ---- attention (2 query-halves to fit PSUM) ----
                for qg in range(2):
                    with (
                        tc.tile_pool(name="otp", bufs=1, space="PSUM") as otp,
                        tc.tile_pool(name="ssp", bufs=3, space="PSUM") as ssp,
                        tc.tile_pool(name="ebp", bufs=3) as ebp,
                        tc.tile_pool(name="esp", bufs=3) as esp,
                        tc.tile_pool(name="ptp", bufs=3) as ptp,
                    ):
                        ots = {}
                        for qc2 in range(2):
                            for b in range(2):
                                ots[(qc2, b)] = otp.tile([65, 512], f32, tag=f"ot{qc2}{b}", name=f"ot{qc2}{b}")
                        for kc in range(16):
                            for qc2 in range(2):
                                qc = qg * 2 + qc2
                                bt = ebp.tile([128, 512], f32, tag="bt")
                                nc.sync.dma_start(
                                    out=bt,
                                    in_=expBT_d.ap()[kc * 128:(kc + 1) * 128, qc * 512:(qc + 1) * 512],
                                )
                                for b in range(2):
                                    q0 = b * N + qc * 512
                                    k0 = b * N + kc * 128
                                    sp = ssp.tile([128, 512], f32, tag="sp")
                                    nc.tensor.matmul(
                                        sp, kT[:, k0:k0 + 128], qT[:, q0:q0 + 512],
                                        start=True, stop=True,
                                    )
                                    es = esp.tile([128, 512], fmm, tag="es")
                                    nc.scalar.activation(es, sp, AF.Exp)
                                    pt = ptp.tile([128, 512], fmm, tag="pt")
                                    nc.vector.tensor_mul(pt, es, bt)
                                    nc.tensor.matmul(
                                        ots[(qc2, b)], vN[:, b * 16 + kc, :], pt,
                                        start=(kc == 0), stop=(kc == 15),
                                    )
                        # drain ot psum -> otT / ogT
                        for qc2 in range(2):
                            for b in range(2):
                                qc = qg * 2 + qc2
                                q0 = b * N + qc * 512
                                nc.vector.tensor_copy(otT[:, q0:q0 + 512], ots[(qc2, b)])
                                nc.vector.tensor_mul(
                                    ogT[:, q0:q0 + 512], otT[0:DH, q0:q0 + 512], gT[:, q0:q0 + 512]
                                )

                # ---- final projection + normalization ----
                with (
                    tc.tile_pool(name="fpp", bufs=3, space="PSUM") as fpp,
                    tc.tile_pool(name="ctp", bufs=2, space="PSUM") as ctp,
                    tc.tile_pool(name="osb", bufs=3) as osb,
                ):
                    for t in range(32):
                        q0 = t * 128
                        tp = ctp.tile([128, 65], f32, tag="ctp")
                        nc.tensor.transpose(tp, otT[:, q0:q0 + 128], ident[0:65, 0:65])
                        nc.vector.reciprocal(recip[:, t:t + 1], tp[:, 64:65])
                        fp = fpp.tile([128, 512], f32, tag="fp")
                        nc.tensor.matmul(fp, ogT[:, q0:q0 + 128], wo_s, start=True, stop=True)
                        ob = osb.tile([128, 512], f32, tag="ob")
                        if t % 2 == 0:
                            nc.scalar.activation(
                                ob, fp, AF.Copy, scale=recip[:, t:t + 1]
                            )
                        else:
                            nc.vector.tensor_scalar_mul(ob, fp, recip[:, t:t + 1])
                        nc.sync.dma_start(out=out_d.ap()[q0:q0 + 128, :], in_=ob)

    nc.compile()
    return nc


def make_in_maps(x, attn_bias, Wq, Wkv, Wo, bo, Wg, bg):
    x = np.asarray(x, dtype=np.float32)
    attn_bias = np.asarray(attn_bias, dtype=np.float32)
    Wq = np.asarray(Wq, dtype=np.float32)
    Wkv = np.asarray(Wkv, dtype=np.float32)
    Wo = np.asarray(Wo, dtype=np.float32)
    Wg = np.asarray(Wg, dtype=np.float32)
    bg = np.asarray(bg, dtype=np.float32)

    xT = np.ascontiguousarray(x.reshape(SEQ, D).T)
    Wk = Wkv[:, :HEADS * DH]
    Wv = Wkv[:, HEADS * DH:]
    in_maps = []
    for h in range(HEADS):
        sl = slice(h * DH, (h + 1) * DH)
        wqk = np.ascontiguousarray(
            np.concatenate([Wq[:, sl] * SCALE, Wk[:, sl]], axis=1))
        wgv = np.ascontiguousarray(
            np.concatenate([Wg[:, sl], Wv[:, sl]], axis=1))
        expBT = np.exp(np.ascontiguousarray(attn_bias[0, h].T))
        in_maps.append({
            "xT": xT,
            "expBT": expBT,
            "wqk": wqk,
            "wgv": wgv,
            "bg": np.ascontiguousarray(bg[sl].reshape(DH, 1)),
            "wo": np.ascontiguousarray(Wo[sl, :]),
        })
    return in_maps


def kernel(x, attn_bias, Wq, Wkv, Wo, bo, Wg, bg):
    from concourse.bass_utils import run_bass_kernel_spmd

    if "nc" not in _CACHE:
        _CACHE["nc"] = build_nc(reps=int(os.environ.get("KERNEL_REPS", "1")))
    nc = _CACHE["nc"]

    in_maps = make_in_maps(x, attn_bias, Wq, Wkv, Wo, bo, Wg, bg)
    res = run_bass_kernel_spmd(nc, in_maps, core_ids=list(range(HEADS)))
    out = np.zeros((SEQ, D), dtype=np.float64)
    for h in range(HEADS):
        out += res.results[h]["out"]
    out += np.asarray(bo, dtype=np.float64)
    return out.astype(np.float32).reshape(B, N, D)
